# revision 10
# baseline (speedup 1.0000x reference)
"""Trainium2 Bass kernel for nn_DecoderRNN: serial LSTM over B*(T+1)=1024 steps
followed by a 32000-vocab softmax head.

Strategy (8 NeuronCores, SPMD single program):
 - The recurrence is inherently serial (state threads through all 1024 steps),
   so every core replicates it: per step, gates = W_hh @ h_{t-1} as 64 bf16
   [128x128]x[128x1] matmuls accumulated in PSUM (the x-projection is
   preloaded into PSUM with an identity matmul). Gate order is [g | i,f | o]
   so the tanh(g) / sigmoid(i,f) activations overlap the tail of the PE
   stream and sigmoid(o) lands last. Cell update is a fused
   [sig_i|sig_f] * [tanh_g|c] multiply + halves-add, then tanh(c) and
   h = sig_o * tanh(c). All gate PSUM tiles are double-buffered so each
   step's matmul burst is contiguous.
 - The per-step matmul burst is weight-load bound (FWL streams W_hh through
   the PE array every step). The PE clock is HAM-throttled to 1.2 GHz unless
   the array stays busy, so ND dummy weight-load/matmul pairs pad the
   PE-idle window while the serial ACT/DVE nonlinearity chain runs,
   keeping the clock gate at 2.4 GHz.
 - x-projection for all steps is one bf16 GEMM done on-device up front.
 - The softmax head is sharded BY STEPS: core c computes full-vocab logits,
   exp and normalization for steps [128c, 128c+128) only (selected via the
   partition-id register with one dynamic-offset copy), writing a
   [128, 32000] fp32 output block. No cross-core communication is needed:
   each core owns complete softmax rows. Host concatenates the 8 blocks.
 - Precision: bf16 for all GEMM inputs and exp storage; fp32 PSUM
   accumulation and cell state throughout.
"""
import sys

if "/opt/trn_rl_repo" not in sys.path:
    sys.path.insert(0, "/opt/trn_rl_repo")

import ml_dtypes
import numpy as np

import concourse.bass as bass
import concourse.tile as tile
from concourse import bacc, mybir

E, H, V = 256, 512, 32000
B, T = 16, 63
S = B * (T + 1)            # 1024 total steps
N_CORES = 8
NW = 500                   # vocab block width
NB = V // NW               # 64 vocab blocks
F32 = mybir.dt.float32
BF16 = mybir.dt.bfloat16
AF = mybir.ActivationFunctionType
ALU = mybir.AluOpType
BF = ml_dtypes.bfloat16

ND = 0                     # dummy filler pairs per step (measured: no help —
FD = 64                    # the weight-load path is NX-clock bound, not HAM)

# gate column groups after the host permutation [g, i, f, o]
# psG = cols 0:4 (g) ; psIF = cols 4:12 (i, f) ; psO = cols 12:16 (o)


def build_nc(steps=S):
    """Build the SPMD Bass program (identical on all cores; the partition-id
    register selects each core's step block in the softmax head)."""
    assert steps % N_CORES == 0
    sblk = steps // N_CORES
    nc = bacc.Bacc("TRN2", target_bir_lowering=False, debug=False,
                   num_devices=N_CORES)

    xsT_d = nc.dram_tensor("xsT", [128, 2, steps], BF16, kind="ExternalInput")
    wihT_d = nc.dram_tensor("wihT", [128, 32, 128], BF16, kind="ExternalInput")
    biasg_d = nc.dram_tensor("biasg", [128, 16], F32, kind="ExternalInput")
    whhT_d = nc.dram_tensor("whhT", [128, 64, 128], BF16, kind="ExternalInput")
    woutT_d = nc.dram_tensor("woutT", [4, 128, V], BF16, kind="ExternalInput")
    bout_d = nc.dram_tensor("bout", [1, V], BF16, kind="ExternalInput")
    ones_d = nc.dram_tensor("ones1", [1, 128], BF16, kind="ExternalInput")
    idn_d = nc.dram_tensor("idn", [128, 128], BF16, kind="ExternalInput")
    probs_d = nc.dram_tensor("probs", [sblk, V], F32, kind="ExternalOutput")

    with tile.TileContext(nc) as tc:
        with tc.tile_pool(name="const", bufs=1) as cpool:
            # ---- persistent SBUF ----
            xsT = cpool.tile([128, 2, steps], BF16)
            wihT = cpool.tile([128, 32, 128], BF16)
            biasg = cpool.tile([128, 16], F32)
            whhT = cpool.tile([128, 64, 128], BF16)
            xprojT = cpool.tile([128, 16, steps], BF16)
            hhist = cpool.tile([128, 4, steps], BF16)
            tgc = cpool.tile([128, 8], F32)       # [tanh(g) | c]
            gact = cpool.tile([128, 12], F32)     # [sig i | sig f | sig o]
            sc = cpool.tile([128, 4], F32)        # sig(2c)
            prod = cpool.tile([128, 8], F32)
            hblk = cpool.tile([128, 4, sblk], BF16)
            ones1 = cpool.tile([1, 128], BF16)
            idn = cpool.tile([128, 128], BF16)
            exps = cpool.tile([128, NB, NW], BF16)
            sums = cpool.tile([128, NB], F32)
            tot = cpool.tile([128, 1], F32)
            inv = cpool.tile([128, 1], F32)

            nc.sync.dma_start(xsT[:], xsT_d.ap())
            nc.sync.dma_start(wihT[:], wihT_d.ap())
            nc.sync.dma_start(biasg[:], biasg_d.ap())
            nc.sync.dma_start(whhT[:], whhT_d.ap())
            nc.sync.dma_start(ones1[:], ones_d.ap())
            nc.sync.dma_start(idn[:], idn_d.ap())
            nc.vector.memset(tgc[:, 4:8], 0.0)    # c0 = 0

            # ---- phase 1: x-projection GEMM (bf16 in, fp32 accum) ----
            nxp = (steps + 511) // 512
            with tc.tile_pool(name="xp_ps", bufs=2, space="PSUM") as xp_ps:
                for j in range(16):
                    for n2 in range(nxp):
                        w = min(512, steps - 512 * n2)
                        ps = xp_ps.tile([128, 512], F32)
                        for e in range(2):
                            nc.tensor.matmul(
                                ps[:, :w],
                                wihT[:, e * 16 + j, :],
                                xsT[:, e, 512 * n2:512 * n2 + w],
                                start=(e == 0), stop=(e == 1))
                        nc.scalar.activation(
                            xprojT[:, j, 512 * n2:512 * n2 + w], ps[:, :w],
                            AF.Identity, bias=biasg[:, j:j + 1])

            # ---- phase 2: serial LSTM recurrence ----
            groups = [(0, 4), (4, 16)]
            with tc.tile_pool(name="g_ps", bufs=2, space="PSUM") as g_ps:
                for t in range(steps):
                    if t == 0:
                        nc.scalar.activation(tgc[:, 0:4], xprojT[:, 0:4, 0],
                                             AF.Tanh)
                        nc.scalar.activation(gact[:, 0:12],
                                             xprojT[:, 4:16, 0], AF.Sigmoid)
                    else:
                        tiles = [g_ps.tile([128, hi - lo], F32,
                                           tag=f"ps{gi}", name=f"ps{gi}_{t}")
                                 for gi, (lo, hi) in enumerate(groups)]
                        # g-group first (its tanh fires mid-burst), then
                        # [i|f|o]; k-major within groups so the first
                        # matmuls of the next step need only h chunk 0
                        for ps, (lo, hi) in zip(tiles, groups):
                            nc.tensor.matmul(ps[:], idn[:],
                                             xprojT[:, lo:hi, t],
                                             start=True, stop=False)
                            for k in range(4):
                                for j in range(lo, hi):
                                    nc.tensor.matmul(
                                        ps[:, j - lo:j - lo + 1],
                                        whhT[:, k * 16 + j, :],
                                        hhist[:, k, t - 1:t],
                                        start=False,
                                        stop=(j == hi - 1 and k == 3))
                        psG, psIFO = tiles
                        nc.scalar.activation(tgc[:, 0:4], psG[:], AF.Tanh)
                        nc.scalar.activation(gact[:, 0:12], psIFO[:],
                                             AF.Sigmoid)
                    # cell update: c = sig_f*c + sig_i*tanh_g, then
                    # h = sig_o * tanh(c); h chunk 0 is written first so the
                    # next step's k0 matmuls can start while chunks 1:4 land
                    nc.vector.tensor_mul(prod[:], gact[:, 0:8], tgc[:, 0:8])
                    nc.vector.tensor_add(tgc[:, 4:8], prod[:, 0:4],
                                         prod[:, 4:8])
                    nc.scalar.activation(sc[:], tgc[:, 4:8], AF.Tanh)
                    nc.vector.tensor_mul(hhist[:, 0, t:t + 1], sc[:, 0:1],
                                         gact[:, 8:9])
                    nc.vector.tensor_mul(hhist[:, 1:4, t], sc[:, 1:4],
                                         gact[:, 9:12])

            # ---- phase 3: per-core step-block softmax head ----
            cid = nc.vector.partition_id()
            off = cid * sblk
            nc.vector.tensor_copy(hblk[:], hhist[:, :, bass.ds(off, sblk)])
            woutT_r = woutT_d.ap().rearrange("k p v -> p k v")
            with tc.tile_pool(name="lg_ps", bufs=2, space="PSUM") as lg_ps, \
                 tc.tile_pool(name="wout", bufs=3) as wpool, \
                 tc.tile_pool(name="bout", bufs=3) as bpool, \
                 tc.tile_pool(name="outstage", bufs=3) as opool:
                for n in range(NB):
                    wt = wpool.tile([128, 4, NW], BF16)
                    nc.sync.dma_start(wt[:], woutT_r[:, :, n * NW:(n + 1) * NW])
                    bt = bpool.tile([1, NW], BF16)
                    nc.sync.dma_start(bt[:], bout_d[0:1, n * NW:(n + 1) * NW])
                    ps = lg_ps.tile([128, NW], F32)
                    nc.tensor.matmul(ps[:sblk, :], ones1[0:1, 0:sblk], bt[:],
                                     start=True, stop=False)
                    for k in range(4):
                        nc.tensor.matmul(ps[:sblk, :], hblk[:, k, :],
                                         wt[:, k, :],
                                         start=False, stop=(k == 3))
                    nc.scalar.activation(exps[:sblk, n, :], ps[:sblk, :],
                                         AF.Exp,
                                         accum_out=sums[:sblk, n:n + 1])
                nc.vector.reduce_sum(tot[:sblk, :], sums[:sblk, :],
                                     axis=mybir.AxisListType.X)
                nc.vector.reciprocal(inv[:sblk, :], tot[:sblk, :])
                for n in range(NB):
                    ot = opool.tile([128, NW], F32)
                    nc.vector.tensor_scalar_mul(ot[:sblk, :],
                                                exps[:sblk, n, :],
                                                inv[:sblk, :])
                    nc.sync.dma_start(probs_d.ap()[:, n * NW:(n + 1) * NW],
                                      ot[:sblk, :])
    nc.compile()
    return nc


def prep_inputs(features, captions, emb, W_ih, W_hh, b_ih, b_hh, W_out, b_out,
                steps=S):
    """Host-side packing: gather + transpose + gate permutation. Pure data
    movement; all FLOPs stay on device."""
    features = np.asarray(features, np.float32)
    captions = np.asarray(captions)
    emb = np.asarray(emb, np.float32)
    W_ih = np.asarray(W_ih, np.float32)
    W_hh = np.asarray(W_hh, np.float32)
    W_out = np.asarray(W_out, np.float32)
    b = np.asarray(b_ih, np.float32) + np.asarray(b_hh, np.float32)
    b_out = np.asarray(b_out, np.float32)

    # gate order [i,f,g,o] -> [g,i,f,o]
    perm = np.concatenate([np.arange(1024, 1536), np.arange(0, 512),
                           np.arange(512, 1024), np.arange(1536, 2048)])
    Wih_p = W_ih[perm]
    Whh_p = W_hh[perm]
    b_p = b[perm]

    xs = np.concatenate([features[:, None, :], emb[captions]], axis=1)
    xs = xs.reshape(S, E)[:steps]
    xsT = np.ascontiguousarray(
        xs.T.reshape(2, 128, steps).transpose(1, 0, 2)).astype(BF)  # [p,e,t]
    wihT = np.ascontiguousarray(
        Wih_p.T.reshape(2, 128, 16, 128).transpose(1, 0, 2, 3)
        .reshape(128, 32, 128)).astype(BF)                        # [p,(e,j),m]
    biasg = np.ascontiguousarray(b_p.reshape(16, 128).T)          # [p,j]
    whhT = np.ascontiguousarray(
        Whh_p.T.reshape(4, 128, 16, 128).transpose(1, 0, 2, 3)
        .reshape(128, 64, 128)).astype(BF)                        # [p,(k,j),m]
    woutT = np.ascontiguousarray(W_out.T.reshape(4, 128, V)).astype(BF)
    bout = b_out[None, :].astype(BF)
    ones1 = np.ones((1, 128), BF)
    idn = np.eye(128, dtype=np.float32).astype(BF)
    return {"xsT": xsT, "wihT": wihT, "biasg": biasg, "whhT": whhT,
            "woutT": woutT, "bout": bout, "ones1": ones1, "idn": idn}


_NC_CACHE = {}


def _get_nc(steps=S):
    if steps not in _NC_CACHE:
        _NC_CACHE[steps] = build_nc(steps)
    return _NC_CACHE[steps]


def kernel(**inputs):
    from concourse.bass_utils import run_bass_kernel_spmd
    nc = _get_nc(S)
    in_map = prep_inputs(**inputs)
    res = run_bass_kernel_spmd(nc, [dict(in_map) for _ in range(N_CORES)],
                               core_ids=list(range(N_CORES)))
    probs = np.concatenate([res.results[c]["probs"] for c in range(N_CORES)],
                           axis=0)
    return probs.reshape(B, T + 1, V).astype(np.float32)
